# revision 22
# baseline (speedup 1.0000x reference)
"""Causal attention forward (out, p_attn) on 8 Trainium2 NeuronCores.

Problem: Q,K,V [32,2048,128] f32, causal mask [32,2048,2048].
  scores = Q@K^T/sqrt(128); masked softmax -> p_attn; out = p_attn@V.
Sharding: 4 heads per core (head-parallel, no cross-core communication).

Per-core kernel design (per head, 16 q-blocks of 128 rows):
  - Q/K blocks are PE-transposed (fp32, exact) into [d, s] layouts QT/KT.
  - S chunk = QT_i^T @ KT (float32r matmuls, 1 cyc/row) into PSUM; the
    causal diagonal 128x128 block gets -1e5 added to its strict upper
    triangle via an extra matmul (NEGL^T trick), so exp underflows to 0.
  - ScalarE exp reads S straight from PSUM with the 1/sqrt(128) scale
    folded in, and emits the per-row sum via accum_out (free rowsum).
  - VectorE: reciprocal + per-row scale -> normalized p tile; DMA out.
    Upper-triangle p_attn stays zero via pre-zeroed output buffers.
  - p blocks are PE-transposed into a per-4-row-chunk pT buffer [k, j, q]
    (f32r), then PV runs V-stationary: outT[d, q512] += V_j^T @ pT_j.
  - outT is PE-transposed back to [q, d] and DMA'd out.
"""
import math
from contextlib import ExitStack

import numpy as np

HB, S_FULL, DK = 32, 2048, 128
N_CORES = 8
HPC = HB // N_CORES  # heads per core
P = 128
CHUNK = 512
SCALE = 1.0 / math.sqrt(float(DK))
NEG_BIG = -1.0e5  # added to raw scores of masked entries; exp underflows to 0

_CACHE = {}
TRACE = False  # set True (e.g. from test.py) to capture an NTFF profile
LAST_RESULTS = None  # BassKernelResults of the most recent run

# tuning knobs (PSUM budget: 2*PS_S + PS_T + PS_V <= 8 banks)
PS_S_BUFS = 2
PS_T_BUFS = 2
PS_V_BUFS = 2
COPY_ACT_OF_16 = 7  # route this many of every 16 PSUM->SBUF copies to ScalarE
SB_BUFS = 2  # double-buffer depth for the big SBUF pools
SM_BUFS = 3  # depth for the small softmax stat tiles
LD_BUFS = 2  # depth for the per-head load pools (qn/kn/vr)
QT_BUFS = 2  # depth for QT/KT pools
PT_BUFS = 2  # depth for the transposed-p buffer
ORDER_MODE = "asc"  # q-block processing order: asc | desc | pair
EP_BUFS = 4  # depth for the exp tiles
P_BUFS = 0   # >0: separate normalized-p pool of this depth; 0: normalize in place


def _build_nc(n_heads, seq):
    import concourse.bacc as bacc
    import concourse.tile as tile
    import concourse.mybir as mybir

    f32 = mybir.dt.float32
    f32r = mybir.dt.float32r
    bf16 = mybir.dt.bfloat16
    EXP = mybir.ActivationFunctionType.Exp
    AX = mybir.AxisListType.X

    NB = seq // P  # number of 128-blocks along seq
    NCK = NB // 4  # number of 4-block chunks (PV granularity)
    assert NB % 4 == 0
    ECHUNK = 1024  # exp granularity (2 PSUM banks per ACT read)

    nc = bacc.Bacc("TRN2", target_bir_lowering=False, debug=False)

    Q_d = nc.dram_tensor("Q", [n_heads, seq, DK], f32, kind="ExternalInput").ap()
    K_d = nc.dram_tensor("K", [n_heads, seq, DK], f32, kind="ExternalInput").ap()
    V_d = nc.dram_tensor("V", [n_heads, seq, DK], f32, kind="ExternalInput").ap()
    IDT_d = nc.dram_tensor("IDT", [P, P], f32, kind="ExternalInput").ap()
    NEGL_d = nc.dram_tensor("NEGL", [P, P], f32, kind="ExternalInput").ap()
    IDTH_d = nc.dram_tensor("IDTH", [P, P], bf16, kind="ExternalInput").ap()
    NEGLH_d = nc.dram_tensor("NEGLH", [P, P], bf16, kind="ExternalInput").ap()
    O_d = nc.dram_tensor("O", [n_heads, seq, DK], f32, kind="ExternalOutput").ap()
    P_d = nc.dram_tensor("P", [n_heads, seq, seq], f32, kind="ExternalOutput").ap()

    with tile.TileContext(nc) as tc, ExitStack() as ctx:
        pool = lambda name, bufs, **kw: ctx.enter_context(
            tc.tile_pool(name=name, bufs=bufs, **kw)
        )
        const_p = pool("const", 1)
        qn_p = pool("qn", LD_BUFS)
        kn_p = pool("kn", LD_BUFS)
        vr_p = pool("vr", LD_BUFS)
        qt_p = pool("qt", QT_BUFS)
        kt_p = pool("kt", QT_BUFS)
        e_p = pool("e", EP_BUFS)
        p_p = pool("p", P_BUFS) if P_BUFS > 0 else None
        pt_p = pool("pt", PT_BUFS)
        ot_p = pool("ot", 2)
        os_p = pool("os", 2)
        # (ot/os small; sm tiny)
        sm_p = pool("sm", SM_BUFS)
        ps_s = pool("ps_s", PS_S_BUFS, space="PSUM")
        ps_t = pool("ps_t", PS_T_BUFS, space="PSUM")
        ps_v = pool("ps_v", PS_V_BUFS, space="PSUM")

        idt = const_p.tile([P, P], f32, tag="idt")
        nc.sync.dma_start(idt[:], IDT_d[:])
        negl = const_p.tile([P, P], bf16, tag="negl")
        nc.sync.dma_start(negl[:], NEGLH_d[:])
        idth = const_p.tile([P, P], bf16, tag="idth")
        nc.sync.dma_start(idth[:], IDTH_d[:])
        copy_idx = [0]

        def psum_copy(dst, src_ap):
            # balance PSUM->SBUF copies between ScalarE and VectorE
            if copy_idx[0] % 16 < COPY_ACT_OF_16:
                nc.scalar.copy(dst, src_ap)
            else:
                nc.vector.tensor_copy(dst, src_ap)
            copy_idx[0] += 1

        for h in range(n_heads):
            vr = vr_p.tile([P, NB, P], f32r, tag="vr")
            nc.sync.dma_start(
                vr[:], V_d[h].rearrange("(j p) d -> p j d", p=P).bitcast(f32r)
            )

            # Build QT/KT [d=128, seq] in f32r via exact fp32 PE transposes.
            qn = qn_p.tile([P, NB, P], f32, tag="qn")
            nc.sync.dma_start(qn[:], Q_d[h].rearrange("(j p) d -> p j d", p=P))
            kn = kn_p.tile([P, NB, P], f32, tag="kn")
            nc.sync.dma_start(kn[:], K_d[h].rearrange("(j p) d -> p j d", p=P))
            qt = qt_p.tile([P, seq], f32r, tag="qt")
            kt = kt_p.tile([P, seq], f32r, tag="kt")
            for src, dst in ((qn, qt), (kn, kt)):
                for g in range(NB // 4):
                    stg = ps_t.tile([P, CHUNK], f32, tag="stg")
                    for m in range(4):
                        nc.tensor.transpose(
                            stg[:, m * P:(m + 1) * P], src[:, g * 4 + m, :], idt[:]
                        )
                    psum_copy(dst[:, g * CHUNK:(g + 1) * CHUNK], stg[:])

            if ORDER_MODE == "pair":
                order = []
                for half in range(NB // 2):
                    order.append(NB - 1 - half)
                    order.append(half)
            elif ORDER_MODE == "desc":
                order = list(range(NB - 1, -1, -1))
            else:
                order = list(range(NB))
            ptbs = {}
            chunk_done = {}
            for i in order:
                W = (i + 1) * P
                nch = (W + ECHUNK - 1) // ECHUNK
                c = i // 4  # PV chunk index
                if c not in ptbs:
                    ptbs[c] = pt_p.tile([P, NB, CHUNK], f32r, tag="ptb", name=f"ptb_h{h}_c{c}")
                ptb = ptbs[c]

                parts = sm_p.tile([P, 2], f32, tag="parts")
                e_t = e_p.tile([P, seq], f32, tag="e")
                for c2 in range(nch):
                    w2 = min(ECHUNK, W - c2 * ECHUNK)
                    sp = ps_s.tile([P, ECHUNK], f32, tag="s")
                    last_e = c2 == nch - 1
                    for m5 in range((w2 + CHUNK - 1) // CHUNK):
                        w3 = min(CHUNK, w2 - m5 * CHUNK)
                        has_diag = last_e and (m5 + 1) * CHUNK >= w2
                        nc.tensor.matmul(
                            sp[:, m5 * CHUNK:m5 * CHUNK + w3],
                            qt[:, i * P:(i + 1) * P],
                            kt[:, c2 * ECHUNK + m5 * CHUNK:
                                c2 * ECHUNK + m5 * CHUNK + w3],
                            start=True,
                            stop=not has_diag,
                        )
                        if has_diag:
                            nc.tensor.matmul(
                                sp[:, w2 - P:w2], negl[:], idth[:],
                                start=False, stop=True,
                            )
                    nc.scalar.activation(
                        e_t[:, c2 * ECHUNK:c2 * ECHUNK + w2],
                        sp[:, :w2],
                        EXP,
                        scale=SCALE,
                        accum_out=parts[:, c2:c2 + 1],
                    )

                rs = sm_p.tile([P, 1], f32, tag="rs")
                nc.vector.reduce_sum(rs[:, 0:1], parts[:, 0:nch], axis=AX)
                rr = sm_p.tile([P, 1], f32, tag="rr")
                nc.vector.reciprocal(rr[:, 0:1], rs[:, 0:1])

                p_t = e_t if p_p is None else p_p.tile([P, seq], f32, tag="p")
                nc.vector.tensor_scalar_mul(p_t[:, :W], e_t[:, :W], rr[:, 0:1])
                nc.sync.dma_start(P_d[h, i * P:(i + 1) * P, 0:W], p_t[:, :W])

                # transpose p blocks into ptb[k, j, q-col of block i]
                for jg in range((i + 4) // 4):
                    jm = min(4, i + 1 - jg * 4)
                    stg = ps_t.tile([P, CHUNK], f32, tag="stg")
                    for m in range(jm):
                        j = jg * 4 + m
                        nc.tensor.transpose(
                            stg[:, m * P:(m + 1) * P], p_t[:, j * P:(j + 1) * P],
                            idt[:],
                        )
                    dst = ptb[:, jg * 4:jg * 4 + jm, (i % 4) * P:(i % 4 + 1) * P]
                    src_ap = stg[:, 0:jm * P].rearrange("p (j q) -> p j q", j=jm)
                    psum_copy(dst, src_ap)

                chunk_done[c] = chunk_done.get(c, 0) + 1
                if chunk_done[c] == 4:
                    del ptbs[c]
                    # PV for q-chunk c: outT[d, q512] = sum_j V_j^T @ pT_j
                    njb = 4 * c + 4
                    ot = ps_v.tile([P, CHUNK], f32, tag="ot")
                    for j in range(njb):
                        qoff = max(0, j - 4 * c) * P
                        nc.tensor.matmul(
                            ot[:, qoff:CHUNK],
                            vr[:, j, :],
                            ptb[:, j, qoff:CHUNK],
                            start=(j == 0),
                            stop=(j == njb - 1),
                            skip_group_check=True,
                        )
                    ot_sb = ot_p.tile([P, CHUNK], f32, tag="ots")
                    psum_copy(ot_sb[:], ot[:])
                    stg = ps_t.tile([P, CHUNK], f32, tag="stg")
                    for m in range(4):
                        nc.tensor.transpose(
                            stg[:, m * P:(m + 1) * P], ot_sb[:, m * P:(m + 1) * P],
                            idt[:],
                        )
                    o_sb = os_p.tile([P, CHUNK], f32, tag="osb")
                    nc.vector.tensor_copy(o_sb[:], stg[:])
                    nc.sync.dma_start(
                        O_d[h, 4 * c * P:(4 * c + 4) * P, :].rearrange(
                            "(m p) d -> p m d", p=P
                        ),
                        o_sb[:].rearrange("p (m d) -> p m d", m=4),
                    )

    nc.compile()
    return nc


def _get_nc(n_heads=HPC, seq=S_FULL):
    key = (n_heads, seq)
    if key not in _CACHE:
        _CACHE[key] = _build_nc(n_heads, seq)
    return _CACHE[key]


def _host_constants():
    import ml_dtypes

    idt = np.eye(P, dtype=np.float32)
    # NEGL[p, q] = NEG_BIG where p > q; matmul adds NEGL^T: S[q, k] += NEGL[k, q]
    negl = np.where(
        np.arange(P)[:, None] > np.arange(P)[None, :], NEG_BIG, 0.0
    ).astype(np.float32)
    return {
        "IDT": idt,
        "NEGL": negl,
        "IDTH": idt.astype(ml_dtypes.bfloat16),
        "NEGLH": negl.astype(ml_dtypes.bfloat16),
    }


def _is_causal(mask):
    if mask.shape != (HB, S_FULL, S_FULL):
        return False
    tril = np.tril(np.ones((S_FULL, S_FULL), dtype=np.float32))
    return bool(np.array_equal(mask, np.broadcast_to(tril, mask.shape)))


def _reference_fallback(Q, K, V, mask):
    out = np.empty_like(Q)
    p_full = np.empty((HB, S_FULL, S_FULL), dtype=np.float32)
    for h in range(HB):
        s = (Q[h].astype(np.float64) @ K[h].T.astype(np.float64)) * SCALE
        s = s * mask[h] + (1.0 - mask[h]) * (-1.0e9)
        s -= s.max(axis=-1, keepdims=True)
        e = np.exp(s)
        p = e / e.sum(axis=-1, keepdims=True)
        p_full[h] = p.astype(np.float32)
        out[h] = (p @ V[h].astype(np.float64)).astype(np.float32)
    return out, p_full


def kernel(Q, K, V, mask):
    from concourse.bass_utils import run_bass_kernel_spmd

    Q = np.ascontiguousarray(Q, dtype=np.float32)
    K = np.ascontiguousarray(K, dtype=np.float32)
    V = np.ascontiguousarray(V, dtype=np.float32)

    if not _is_causal(np.asarray(mask)):
        return _reference_fallback(Q, K, V, np.asarray(mask, dtype=np.float32))

    nc = _get_nc()
    consts = _host_constants()
    in_maps = []
    for core in range(N_CORES):
        sl = slice(core * HPC, (core + 1) * HPC)
        in_maps.append({"Q": Q[sl], "K": K[sl], "V": V[sl], **consts})
    global LAST_RESULTS
    try:
        res = run_bass_kernel_spmd(
            nc, in_maps, list(range(N_CORES)), trace=TRACE
        )
    except Exception:
        # transient NRT execution faults happen occasionally; retry once
        res = run_bass_kernel_spmd(
            nc, in_maps, list(range(N_CORES)), trace=TRACE
        )
    LAST_RESULTS = res
    out = np.concatenate([r["O"] for r in res.results], axis=0)
    p_attn = np.concatenate([r["P"] for r in res.results], axis=0)
    return out, p_attn


# revision 30
# speedup vs baseline: 35.7483x; 35.7483x over previous
"""Causal attention forward (out, p_attn) on 8 Trainium2 NeuronCores.

Problem: Q,K,V [32,2048,128] f32, causal mask [32,2048,2048].
  scores = Q@K^T/sqrt(128); masked softmax -> p_attn; out = p_attn@V.
Sharding: 4 heads per core (head-parallel, no cross-core communication).

Per-core kernel design (per head, 16 q-blocks of 128 rows):
  - Q/K blocks are PE-transposed (fp32, exact) into [d, s] layouts QT/KT.
  - S chunk = QT_i^T @ KT (float32r matmuls, 1 cyc/row) into PSUM; the
    causal diagonal 128x128 block gets -1e5 added to its strict upper
    triangle via an extra matmul (NEGL^T trick), so exp underflows to 0.
  - ScalarE exp reads S straight from PSUM with the 1/sqrt(128) scale
    folded in, and emits the per-row sum via accum_out (free rowsum).
  - VectorE: reciprocal + per-row scale -> normalized p tile; DMA out.
    Upper-triangle p_attn stays zero via pre-zeroed output buffers.
  - p blocks are PE-transposed into a per-4-row-chunk pT buffer [k, j, q]
    (f32r), then PV runs V-stationary: outT[d, q512] += V_j^T @ pT_j.
  - outT is PE-transposed back to [q, d] and DMA'd out.
"""
import math
from contextlib import ExitStack

import numpy as np

HB, S_FULL, DK = 32, 2048, 128
N_CORES = 8
HPC = HB // N_CORES  # heads per core
P = 128
CHUNK = 512
SCALE = 1.0 / math.sqrt(float(DK))
NEG_BIG = -1.0e5  # added to raw scores of masked entries; exp underflows to 0

_CACHE = {}
TRACE = False  # set True (e.g. from test.py) to capture an NTFF profile
LAST_RESULTS = None  # BassKernelResults of the most recent run

# tuning knobs (PSUM budget: 2*PS_S + PS_T + PS_V <= 8 banks)
PS_S_BUFS = 2
PS_T_BUFS = 3
PS_V_BUFS = 1
COPY_ACT_OF_16 = 5  # route this many of every 16 PSUM->SBUF copies to ScalarE
SB_BUFS = 2  # double-buffer depth for the big SBUF pools
SM_BUFS = 3  # depth for the small softmax stat tiles
LD_BUFS = 2  # depth for the per-head load pools (qn/kn/vr)
QT_BUFS = 2  # depth for QT/KT pools
PT_BUFS = 2  # depth for the transposed-p buffer
ORDER_MODE = "asc"  # q-block processing order: asc | desc | pair
OT_BUFS = 1  # outT staging depth
HEAD_EMIT_LEFT = 10  # emit next-head build when this many blocks remain
SPLIT_NORM = False  # split p normalize+DMA into halves for wide rows
EP_BUFS = 3  # depth for the exp tiles
P_BUFS = 4   # >0: separate normalized-p pool of this depth; 0: normalize in place


def _build_nc(n_heads, seq):
    import concourse.bacc as bacc
    import concourse.tile as tile
    import concourse.mybir as mybir

    f32 = mybir.dt.float32
    f32r = mybir.dt.float32r
    bf16 = mybir.dt.bfloat16
    EXP = mybir.ActivationFunctionType.Exp
    AX = mybir.AxisListType.X

    NB = seq // P  # number of 128-blocks along seq
    NCK = NB // 4  # number of 4-block chunks (PV granularity)
    assert NB % 4 == 0
    ECHUNK = 1024  # exp granularity (2 PSUM banks per ACT read)

    nc = bacc.Bacc("TRN2", target_bir_lowering=False, debug=False)

    Q_d = nc.dram_tensor("Q", [n_heads, seq, DK], f32, kind="ExternalInput").ap()
    K_d = nc.dram_tensor("K", [n_heads, seq, DK], f32, kind="ExternalInput").ap()
    V_d = nc.dram_tensor("V", [n_heads, seq, DK], f32, kind="ExternalInput").ap()
    IDT_d = nc.dram_tensor("IDT", [P, P], f32, kind="ExternalInput").ap()
    NEGL_d = nc.dram_tensor("NEGL", [P, P], f32, kind="ExternalInput").ap()
    IDTH_d = nc.dram_tensor("IDTH", [P, P], bf16, kind="ExternalInput").ap()
    NEGLH_d = nc.dram_tensor("NEGLH", [P, P], bf16, kind="ExternalInput").ap()
    O_d = nc.dram_tensor("O", [n_heads, seq, DK], f32, kind="ExternalOutput").ap()
    P_d = nc.dram_tensor("P", [n_heads, seq, seq], f32, kind="ExternalOutput").ap()

    with tile.TileContext(nc) as tc, ExitStack() as ctx:
        pool = lambda name, bufs, **kw: ctx.enter_context(
            tc.tile_pool(name=name, bufs=bufs, **kw)
        )
        const_p = pool("const", 1)
        qn_p = pool("qn", LD_BUFS)
        kn_p = pool("kn", LD_BUFS)
        vr_p = pool("vr", LD_BUFS)
        qt_p = pool("qt", QT_BUFS)
        kt_p = pool("kt", QT_BUFS)
        e_p = pool("e", EP_BUFS)
        p_p = pool("p", P_BUFS) if P_BUFS > 0 else None
        pt_p = pool("pt", PT_BUFS)
        ot_p = pool("ot", OT_BUFS)
        os_p = pool("os", 2)
        # (ot/os small; sm tiny)
        sm_p = pool("sm", SM_BUFS)
        ps_s = pool("ps_s", PS_S_BUFS, space="PSUM")
        ps_t = pool("ps_t", PS_T_BUFS, space="PSUM")
        ps_v = pool("ps_v", PS_V_BUFS, space="PSUM")

        idt = const_p.tile([P, P], f32, tag="idt")
        nc.sync.dma_start(idt[:], IDT_d[:])
        negl = const_p.tile([P, P], bf16, tag="negl")
        nc.sync.dma_start(negl[:], NEGLH_d[:])
        idth = const_p.tile([P, P], bf16, tag="idth")
        nc.sync.dma_start(idth[:], IDTH_d[:])
        idtr = const_p.tile([P, P], f32r, tag="idtr")
        nc.sync.dma_start(idtr[:], IDT_d[:].bitcast(f32r))
        copy_idx = [0]

        def psum_copy(dst, src_ap):
            # balance PSUM->SBUF copies between ScalarE and VectorE
            if copy_idx[0] % 16 < COPY_ACT_OF_16:
                nc.scalar.copy(dst, src_ap)
            else:
                nc.vector.tensor_copy(dst, src_ap)
            copy_idx[0] += 1

        def emit_head_build(h):
            # load V + build QT/KT [d=128, seq] (f32r) via PE transposes
            vr = vr_p.tile([P, NB, P], f32r, tag="vr", name=f"vr{h}")
            nc.sync.dma_start(
                vr[:], V_d[h].rearrange("(j p) d -> p j d", p=P).bitcast(f32r)
            )
            qn = qn_p.tile([P, NB, P], f32r, tag="qn", name=f"qn{h}")
            nc.sync.dma_start(
                qn[:], Q_d[h].rearrange("(j p) d -> p j d", p=P).bitcast(f32r)
            )
            kn = kn_p.tile([P, NB, P], f32r, tag="kn", name=f"kn{h}")
            nc.sync.dma_start(
                kn[:], K_d[h].rearrange("(j p) d -> p j d", p=P).bitcast(f32r)
            )
            qt = qt_p.tile([P, seq], f32r, tag="qt", name=f"qt{h}")
            kt = kt_p.tile([P, seq], f32r, tag="kt", name=f"kt{h}")
            for src, dst in ((qn, qt), (kn, kt)):
                for g in range(NB // 4):
                    stg = ps_t.tile([P, CHUNK], f32, tag="stg")
                    for m in range(4):
                        nc.tensor.transpose(
                            stg[:, m * P:(m + 1) * P].bitcast(f32r),
                            src[:, g * 4 + m, :], idtr[:],
                        )
                    psum_copy(dst[:, g * CHUNK:(g + 1) * CHUNK], stg[:])
            return vr, qt, kt

        built = {0: emit_head_build(0)}
        for h in range(n_heads):
            vr, qt, kt = built.pop(h)

            if ORDER_MODE == "pair":
                order = []
                for half in range(NB // 2):
                    order.append(NB - 1 - half)
                    order.append(half)
            elif ORDER_MODE == "desc":
                order = list(range(NB - 1, -1, -1))
            else:
                order = list(range(NB))
            ptbs = {}
            chunk_done = {}
            n_done = 0
            for i in order:
                n_done += 1
                W = (i + 1) * P
                nch = (W + ECHUNK - 1) // ECHUNK
                c = i // 4  # PV chunk index
                if c not in ptbs:
                    ptbs[c] = pt_p.tile([P, NB, CHUNK], f32r, tag="ptb", name=f"ptb_h{h}_c{c}")
                ptb = ptbs[c]

                parts = sm_p.tile([P, 2], f32, tag="parts")
                e_t = e_p.tile([P, seq], f32, tag="e")
                for c2 in range(nch):
                    w2 = min(ECHUNK, W - c2 * ECHUNK)
                    sp = ps_s.tile([P, ECHUNK], f32, tag="s")
                    last_e = c2 == nch - 1
                    for m5 in range((w2 + CHUNK - 1) // CHUNK):
                        w3 = min(CHUNK, w2 - m5 * CHUNK)
                        has_diag = last_e and (m5 + 1) * CHUNK >= w2
                        nc.tensor.matmul(
                            sp[:, m5 * CHUNK:m5 * CHUNK + w3],
                            qt[:, i * P:(i + 1) * P],
                            kt[:, c2 * ECHUNK + m5 * CHUNK:
                                c2 * ECHUNK + m5 * CHUNK + w3],
                            start=True,
                            stop=not has_diag,
                        )
                        if has_diag:
                            nc.tensor.matmul(
                                sp[:, w2 - P:w2], negl[:], idth[:],
                                start=False, stop=True,
                            )
                    nc.scalar.activation(
                        e_t[:, c2 * ECHUNK:c2 * ECHUNK + w2],
                        sp[:, :w2],
                        EXP,
                        scale=SCALE,
                        accum_out=parts[:, c2:c2 + 1],
                    )

                rr = sm_p.tile([P, 1], f32, tag="rr")
                if nch == 1:
                    nc.vector.reciprocal(rr[:, 0:1], parts[:, 0:1])
                else:
                    rs = sm_p.tile([P, 1], f32, tag="rs")
                    nc.vector.reduce_sum(rs[:, 0:1], parts[:, 0:nch], axis=AX)
                    nc.vector.reciprocal(rr[:, 0:1], rs[:, 0:1])

                p_t = e_t if p_p is None else p_p.tile([P, seq], f32r, tag="p")
                if SPLIT_NORM and W >= 1024:
                    half = (W // 2) // P * P
                    for lo, hi in ((0, half), (half, W)):
                        nc.vector.tensor_scalar_mul(
                            p_t[:, lo:hi], e_t[:, lo:hi], rr[:, 0:1]
                        )
                        nc.sync.dma_start(
                            P_d[h, i * P:(i + 1) * P, lo:hi].bitcast(f32r),
                            p_t[:, lo:hi],
                        )
                else:
                    nc.vector.tensor_scalar_mul(p_t[:, :W], e_t[:, :W], rr[:, 0:1])
                    nc.sync.dma_start(
                        P_d[h, i * P:(i + 1) * P, 0:W].bitcast(f32r), p_t[:, :W]
                    )

                # transpose p blocks into ptb[k, j, q-col of block i]
                for jg in range((i + 4) // 4):
                    jm = min(4, i + 1 - jg * 4)
                    stg = ps_t.tile([P, CHUNK], f32, tag="stg")
                    for m in range(jm):
                        j = jg * 4 + m
                        nc.tensor.transpose(
                            stg[:, m * P:(m + 1) * P].bitcast(f32r),
                            p_t[:, j * P:(j + 1) * P], idtr[:],
                        )
                    dst = ptb[:, jg * 4:jg * 4 + jm, (i % 4) * P:(i % 4 + 1) * P]
                    src_ap = stg[:, 0:jm * P].rearrange("p (j q) -> p j q", j=jm)
                    psum_copy(dst, src_ap)

                chunk_done[c] = chunk_done.get(c, 0) + 1
                if n_done == NB - HEAD_EMIT_LEFT and h + 1 < n_heads:
                    built[h + 1] = emit_head_build(h + 1)
                if chunk_done[c] == 4:
                    del ptbs[c]
                    # PV for q-chunk c: outT[d, q512] = sum_j V_j^T @ pT_j
                    njb = 4 * c + 4
                    ot = ps_v.tile([P, CHUNK], f32, tag="ot")
                    for j in range(njb):
                        qoff = max(0, j - 4 * c) * P
                        nc.tensor.matmul(
                            ot[:, qoff:CHUNK],
                            vr[:, j, :],
                            ptb[:, j, qoff:CHUNK],
                            start=(j == 0),
                            stop=(j == njb - 1),
                            skip_group_check=True,
                        )
                    ot_sb = ot_p.tile([P, CHUNK], f32, tag="ots")
                    psum_copy(ot_sb[:], ot[:])
                    stg = ps_t.tile([P, CHUNK], f32, tag="stg")
                    for m in range(4):
                        nc.tensor.transpose(
                            stg[:, m * P:(m + 1) * P], ot_sb[:, m * P:(m + 1) * P],
                            idt[:],
                        )
                    o_sb = os_p.tile([P, CHUNK], f32, tag="osb")
                    nc.vector.tensor_copy(o_sb[:], stg[:])
                    nc.sync.dma_start(
                        O_d[h, 4 * c * P:(4 * c + 4) * P, :].rearrange(
                            "(m p) d -> p m d", p=P
                        ),
                        o_sb[:].rearrange("p (m d) -> p m d", m=4),
                    )

    nc.compile()
    return nc


def _get_nc(n_heads=HPC, seq=S_FULL):
    key = (n_heads, seq)
    if key not in _CACHE:
        _CACHE[key] = _build_nc(n_heads, seq)
    return _CACHE[key]


def _host_constants():
    import ml_dtypes

    idt = np.eye(P, dtype=np.float32)
    # NEGL[p, q] = NEG_BIG where p > q; matmul adds NEGL^T: S[q, k] += NEGL[k, q]
    negl = np.where(
        np.arange(P)[:, None] > np.arange(P)[None, :], NEG_BIG, 0.0
    ).astype(np.float32)
    return {
        "IDT": idt,
        "NEGL": negl,
        "IDTH": idt.astype(ml_dtypes.bfloat16),
        "NEGLH": negl.astype(ml_dtypes.bfloat16),
    }


def _is_causal(mask):
    if mask.shape != (HB, S_FULL, S_FULL):
        return False
    tril = np.tril(np.ones((S_FULL, S_FULL), dtype=np.float32))
    return bool(np.array_equal(mask, np.broadcast_to(tril, mask.shape)))


def _reference_fallback(Q, K, V, mask):
    out = np.empty_like(Q)
    p_full = np.empty((HB, S_FULL, S_FULL), dtype=np.float32)
    for h in range(HB):
        s = (Q[h].astype(np.float64) @ K[h].T.astype(np.float64)) * SCALE
        s = s * mask[h] + (1.0 - mask[h]) * (-1.0e9)
        s -= s.max(axis=-1, keepdims=True)
        e = np.exp(s)
        p = e / e.sum(axis=-1, keepdims=True)
        p_full[h] = p.astype(np.float32)
        out[h] = (p @ V[h].astype(np.float64)).astype(np.float32)
    return out, p_full


def kernel(Q, K, V, mask):
    from concourse.bass_utils import run_bass_kernel_spmd

    Q = np.ascontiguousarray(Q, dtype=np.float32)
    K = np.ascontiguousarray(K, dtype=np.float32)
    V = np.ascontiguousarray(V, dtype=np.float32)

    if not _is_causal(np.asarray(mask)):
        return _reference_fallback(Q, K, V, np.asarray(mask, dtype=np.float32))

    nc = _get_nc()
    consts = _host_constants()
    in_maps = []
    for core in range(N_CORES):
        sl = slice(core * HPC, (core + 1) * HPC)
        in_maps.append({"Q": Q[sl], "K": K[sl], "V": V[sl], **consts})
    global LAST_RESULTS
    try:
        res = run_bass_kernel_spmd(
            nc, in_maps, list(range(N_CORES)), trace=TRACE
        )
    except Exception:
        # transient NRT execution faults happen occasionally; retry once
        res = run_bass_kernel_spmd(
            nc, in_maps, list(range(N_CORES)), trace=TRACE
        )
    LAST_RESULTS = res
    out = np.concatenate([r["O"] for r in res.results], axis=0)
    p_attn = np.concatenate([r["P"] for r in res.results], axis=0)
    return out, p_attn
